# revision 10
# baseline (speedup 1.0000x reference)
"""Trainium2 Bass kernel for nn_BaseLoftqLinear (4-bit quantized linear + LoRA + bias).

Computes: out = x @ dequant(W).T + (x @ A.T) @ B.T + bias
  x: [4, 2048, 4096] f32, W: [4096, 4096] 4-bit packed, A: [16, 4096], B: [4096, 16]

Strategy (column-parallel over out_features across 8 cores):
  - each core owns 512 out_features: its shard of packed_qweight/weight_max/lora_B/bias
  - x replicated; shipped k-major (transposed) in bf16 so xT k-chunk tiles load
    with plain contiguous DMAs (no on-device transpose of x at all)
  - on device: dequantize W shard on DVE (nibble extract, fused (nib+delta)*scale,
    add bf16 B@A lora product), PE-transpose W_eff chunk-major, stream the big
    bf16 GEMM x @ W_eff.T, add bias on DVE
  - first 6 m-tiles run chunk-outer so the PE consumes W_eff chunks as the
    dequant stream produces them instead of waiting for all of W-prep
  - host gathers the 8 [8192, 512] outputs -> [4, 2048, 4096]
"""
import os
import sys

for _p in ("/opt/trn_rl_repo", "/root/.axon_site/_ro/trn_rl_repo"):
    if os.path.isdir(_p) and _p not in sys.path:
        sys.path.insert(0, _p)
        break

import numpy as np
import ml_dtypes

import concourse.bass as bass
import concourse.bacc as bacc
import concourse.tile as tile
import concourse.mybir as mybir

dt = mybir.dt

# problem constants (hardcoded per spec)
B_, S_, IN_F, OUT_F, RANK = 4, 2048, 4096, 4096, 16
N_CORES = 8
M = B_ * S_                    # 8192 tokens
N = OUT_F // N_CORES           # 512 out_features per core
BLOCK = 64                     # quant block size (along in_features)
NB = IN_F // 2                 # 2048 packed bytes per out_feature row
OT = N // 128                  # 4 o-tiles per core
MT = M // 128                  # 64 m-tiles
KC = IN_F // 128               # 32 k-chunks
SM = 512                       # m-strip size for x loads
NS = M // SM                   # 16 strips
MPS = SM // 128                # 4 m-tiles per strip
PIPE = 5                       # m-tiles processed chunk-outer during W-prep


def build_program(affine: bool, c1: float, delta: float, lut_vals):
    """Build the single-core Bass program (SPMD: same program on all 8 cores)."""
    nc = bacc.Bacc("TRN2", target_bir_lowering=False, debug=False,
                   num_devices=N_CORES)

    xt_d = nc.dram_tensor("xt", [IN_F, M], dt.bfloat16, kind="ExternalInput")
    pk = nc.dram_tensor("pk", [N, NB], dt.uint8, kind="ExternalInput")
    wmax = nc.dram_tensor("wmax", [N, BLOCK], dt.float32, kind="ExternalInput")
    lora_a = nc.dram_tensor("lora_a", [RANK, IN_F], dt.bfloat16, kind="ExternalInput")
    lora_bt = nc.dram_tensor("lora_bt", [RANK, N], dt.bfloat16, kind="ExternalInput")
    bias = nc.dram_tensor("bias", [N], dt.float32, kind="ExternalInput")
    ident = nc.dram_tensor("ident", [128, 128], dt.bfloat16, kind="ExternalInput")
    out = nc.dram_tensor("out", [M, N], dt.float32, kind="ExternalOutput")

    with tile.TileContext(nc) as tc:
        with (
            tc.tile_pool(name="const", bufs=1) as constp,
            tc.tile_pool(name="wprep", bufs=2) as wprep,
            tc.tile_pool(name="wsb", bufs=2) as wsbp,
            tc.tile_pool(name="wt", bufs=1) as wtp,
            tc.tile_pool(name="xt", bufs=3) as xtp,
            tc.tile_pool(name="osb", bufs=3) as op_,
            tc.tile_pool(name="ps_ba", bufs=2, space="PSUM") as ps_ba,
            tc.tile_pool(name="ps_tx", bufs=1, space="PSUM") as ps_tx,
            tc.tile_pool(name="ps_out", bufs=1, space="PSUM") as ps_out,
        ):
            # ---- x strips: plain contiguous DMA from k-major xT ----
            xt_tiles = {}

            def load_strip(s):
                for c in range(KC):
                    xt = xtp.tile([128, SM], dt.bfloat16, tag=f"xt{c}")
                    eng = nc.sync if (c % 2 == 0) else nc.scalar
                    eng.dma_start(
                        out=xt[:],
                        in_=xt_d[c * 128:(c + 1) * 128, s * SM:(s + 1) * SM],
                    )
                    xt_tiles[(s, c)] = xt

            # ---- constants + W bytes first (small; must not queue behind
            # the 8.4MB x-strip preloads on the DMA rings) ----
            a_sb = constp.tile([RANK, IN_F], dt.bfloat16, name="a_sb")
            nc.gpsimd.dma_start(out=a_sb[:], in_=lora_a[:, :])
            bt_sb = constp.tile([RANK, N], dt.bfloat16, name="bt_sb")
            nc.gpsimd.dma_start(out=bt_sb[:], in_=lora_bt[:, :])
            id_sb = constp.tile([128, 128], dt.bfloat16, name="id_sb")
            nc.gpsimd.dma_start(out=id_sb[:], in_=ident[:, :])

            # W bytes + scales (per o-tile)
            bt_u8s, s1s = [], []
            for t in range(OT):
                bt_u8 = wprep.tile([128, NB], dt.uint8, tag=f"bytes{t}", bufs=1)
                nc.gpsimd.dma_start(out=bt_u8[:], in_=pk[t * 128:(t + 1) * 128, :])
                s1 = wprep.tile([128, BLOCK], dt.float32, tag=f"s1{t}", bufs=1)
                nc.gpsimd.dma_start(out=s1[:], in_=wmax[t * 128:(t + 1) * 128, :])
                if affine:
                    nc.vector.tensor_scalar_mul(s1[:], s1[:], float(c1))
                bt_u8s.append(bt_u8)
                s1s.append(s1)

            bias_sb = constp.tile([128, N], dt.float32, name="bias_sb")
            bsrc = bass.AP(bias[:].tensor, 0, [[0, 128], [1, N]])
            nc.gpsimd.dma_start(out=bias_sb[:], in_=bsrc)

            load_strip(0)
            load_strip(1)

            # ---- W-prep, chunk-major: dequant + B@A fused -> PE transpose ----
            # wt_sb[k_p, c*N + o] = W_eff[o, c*128 + k_p]
            wt_sb = wtp.tile([128, KC * N], dt.bfloat16, name="wt_sb")

            NCH = 8
            CW = IN_F // NCH          # 512 in_f per chunk group
            CB = CW // 2              # 256 bytes per chunk group
            for ch in range(NCH):
                w_chunks = []
                for t in range(OT):
                    # lora B@A for this [128 o, 512 in] chunk (bf16, K=16)
                    pba = ps_ba.tile([128, CW], dt.float32, tag="pba")
                    nc.tensor.matmul(
                        pba[:], bt_sb[:, t * 128:(t + 1) * 128],
                        a_sb[:, ch * CW:(ch + 1) * CW],
                        start=True, stop=True,
                    )
                    w_sb = wsbp.tile([128, CW], dt.bfloat16, tag=f"w{t}")
                    w_chunks.append(w_sb)
                    by = bt_u8s[t][:, ch * CB:(ch + 1) * CB]
                    s_off = ch * (CW // BLOCK)
                    s_ap0 = s1s[t][:]
                    for half, (op0, arg0) in enumerate(
                        ((mybir.AluOpType.bitwise_and, 15),
                         (mybir.AluOpType.logical_shift_right, 4))
                    ):
                        nib = wprep.tile([128, CB], dt.uint8, tag="deq_nib")
                        nc.vector.tensor_scalar(nib[:], by, arg0, None, op0)
                        # per-block scale, broadcast 32 bytes/block
                        s_b = bass.AP(
                            s_ap0.tensor, s_ap0.offset + s_off,
                            [list(s_ap0.ap[0]), [1, CW // BLOCK], [0, BLOCK // 2]],
                        )
                        tl = wprep.tile([128, CB], dt.float32, tag="deq_t")
                        if affine:
                            # (nib + delta) * scale in one fused DVE op
                            nc.vector.scalar_tensor_tensor(
                                tl[:], nib[:], float(delta), s_b,
                                mybir.AluOpType.add, mybir.AluOpType.mult)
                        else:
                            # general 16-entry codebook fallback:
                            # idx -> sum_k lut[k] * (idx == k)
                            nc.vector.memset(tl[:], 0.0)
                            for k in range(16):
                                msk = wprep.tile([128, CB], dt.float32, tag="deq_msk")
                                nc.vector.tensor_scalar(
                                    msk[:], nib[:], float(k), None,
                                    mybir.AluOpType.is_equal,
                                )
                                nc.vector.tensor_scalar_mul(
                                    msk[:], msk[:], float(lut_vals[k]))
                                nc.vector.tensor_tensor(
                                    tl[:], tl[:], msk[:], mybir.AluOpType.add)
                            nc.vector.tensor_tensor(
                                tl[:], tl[:], s_b, mybir.AluOpType.mult)
                        # add B@A (strided: half-th of each in_f pair) -> bf16
                        pba_ap = pba[:]
                        ba_s = bass.AP(
                            pba_ap.tensor, pba_ap.offset + half,
                            [list(pba_ap.ap[0]), [2, CB]],
                        )
                        w_ap = w_sb[:]
                        w_dst = bass.AP(
                            w_ap.tensor, w_ap.offset + half,
                            [list(w_ap.ap[0]), [2, CB]],
                        )
                        nc.vector.tensor_tensor(w_dst, tl[:], ba_s, mybir.AluOpType.add)

                # PE-transpose the 4 k-chunks of this group; evict on ACT
                for cc in range(CW // 128):
                    c = ch * (CW // 128) + cc
                    ptr = ps_tx.tile([128, N], dt.bfloat16, tag="ptx")
                    for t in range(OT):
                        nc.tensor.transpose(
                            ptr[:, t * 128:(t + 1) * 128],
                            w_chunks[t][:, cc * 128:(cc + 1) * 128],
                            id_sb[:],
                        )
                    nc.scalar.activation(
                        wt_sb[:, c * N:(c + 1) * N], ptr[:],
                        mybir.ActivationFunctionType.Copy, bias=0.0)

            # ---- main loop ----
            def evict(ms, po):
                o_sb = op_.tile([128, N], dt.float32, tag="o_sb")
                nc.vector.tensor_tensor(
                    o_sb[:], po[:], bias_sb[:], mybir.AluOpType.add)
                nc.gpsimd.dma_start(
                    out=out[ms * 128:(ms + 1) * 128, :], in_=o_sb[:])

            def xt_of(ms, c):
                return xt_tiles[(ms // MPS, c)][:, (ms % MPS) * 128:
                                                (ms % MPS + 1) * 128]

            # first PIPE m-tiles chunk-outer: consume wt chunks as W-prep
            # streams them, so the PE never waits for the whole dequant
            load_strip(2)
            pos = [ps_out.tile([128, N], dt.float32, tag=f"po{i}", name=f"po{i}")
                   for i in range(PIPE)]
            for c in range(KC):
                for i in range(PIPE):
                    nc.tensor.matmul(
                        pos[i][:], xt_of(i, c), wt_sb[:, c * N:(c + 1) * N],
                        start=(c == 0), stop=(c == KC - 1),
                    )
            for i in range(PIPE):
                evict(i, pos[i])

            # rest m-outer
            loaded = 3  # strips 0,1,2 already issued
            for ms in range(PIPE, MT):
                if ms % MPS == 0:
                    s = ms // MPS
                    while loaded <= s + 2 and loaded < NS:
                        load_strip(loaded)
                        loaded += 1
                po = ps_out.tile([128, N], dt.float32, tag=f"po{ms % PIPE}",
                                 name=f"po{ms % PIPE}")
                for c in range(KC):
                    nc.tensor.matmul(
                        po[:], xt_of(ms, c), wt_sb[:, c * N:(c + 1) * N],
                        start=(c == 0), stop=(c == KC - 1),
                    )
                evict(ms, po)

    nc.compile()
    return nc


_cache = {}


def _get_program(lut: np.ndarray):
    lut = np.asarray(lut, dtype=np.float32)
    c1 = float(lut[15] - lut[0]) / 15.0
    idx = np.arange(16, dtype=np.float32)
    affine = bool(
        np.max(np.abs(lut - (lut[0] + c1 * idx))) <= 1e-6 * max(1e-30, np.max(np.abs(lut)))
        and abs(c1) > 1e-20
    )
    delta = float(lut[0]) / c1 if affine else 0.0
    key = (affine, round(c1, 12), round(delta, 12), tuple(np.round(lut, 10).tolist()))
    if key not in _cache:
        _cache[key] = build_program(affine, c1, delta, lut.tolist())
    return _cache[key]


def make_in_maps(inputs: dict):
    x = np.asarray(inputs["x"], dtype=np.float32).reshape(M, IN_F)
    xt = np.ascontiguousarray(x.astype(ml_dtypes.bfloat16).T)  # [IN_F, M]
    pk_full = np.asarray(inputs["packed_qweight"]).astype(np.uint8).reshape(-1)
    wmax_full = np.asarray(inputs["weight_max"], dtype=np.float32).reshape(-1)
    lora_a = np.ascontiguousarray(
        np.asarray(inputs["lora_A"], dtype=np.float32).astype(ml_dtypes.bfloat16))
    lora_b = np.asarray(inputs["lora_B"], dtype=np.float32)
    bias_full = np.asarray(inputs["bias"], dtype=np.float32).reshape(-1)
    ident = np.eye(128, dtype=ml_dtypes.bfloat16)

    in_maps = []
    for i in range(N_CORES):
        o0, o1 = i * N, (i + 1) * N
        in_maps.append({
            "xt": xt,
            "pk": pk_full[o0 * NB: o1 * NB].reshape(N, NB),
            "wmax": wmax_full[o0 * BLOCK: o1 * BLOCK].reshape(N, BLOCK),
            "lora_a": lora_a,
            "lora_bt": np.ascontiguousarray(
                lora_b[o0:o1].T.astype(ml_dtypes.bfloat16)),
            "bias": bias_full[o0:o1],
            "ident": ident,
        })
    return in_maps


def kernel(**inputs) -> np.ndarray:
    from concourse.bass_utils import run_bass_kernel_spmd

    nc = _get_program(inputs["lookup_table"])
    in_maps = make_in_maps(inputs)
    res = run_bass_kernel_spmd(nc, in_maps, core_ids=list(range(N_CORES)))
    outs = [np.asarray(r["out"], dtype=np.float32) for r in res.results]
    full = np.concatenate(outs, axis=1)  # [M, OUT_F]
    return full.reshape(B_, S_, OUT_F)
